# revision 1
# baseline (speedup 1.0000x reference)
"""Gaussian KDE (bandwidth=0.5) on 8 TRN2 NeuronCores.

out[j] = sum_i mask_i * exp(-|s_i - l_j|^2 / bw^2), normalized to sum 1.

Strategy (data-parallel over samples):
  - core c gets samples[c*2048:(c+1)*2048] and all 8192 locations.
  - exp argument is expanded as a K=3 matmul:
        arg[p,i] = 8*(lx_j*sx_i + ly_j*sy_i) + t_i + bias_j
    with stationary lhsT = [lx; ly; 1] (per 128-location block),
    moving rhs = [8*sx; 8*sy; t],  t_i = -4*|s_i|^2 + 500*(inx_i+iny_i),
    bias_j = -4*|l_j|^2 - 1000  (ACT per-partition bias).
    For in-bbox samples (inx+iny==2) this is exactly -4*|s-l|^2; otherwise
    it is <= -500 and exp underflows to exactly 0 (torch mask semantics).
  - ScalarE ACT computes exp over each [128, 2048] PSUM tile with a fused
    free-axis accumulate (accum_out) -> per-core partial sums [128, 64].
  - AllReduce over the 8 cores, then each core normalizes on-device.

Location index mapping: j = p*64 + b (partition p, block b), so the final
[128, 64] SBUF accumulator stores row-major j and the output DMA is
contiguous.
"""

import sys

sys.path.insert(0, "/opt/trn_rl_repo")

import numpy as np

N_CORES = 8
NS = 16384
NL = 8192
NS_SH = NS // N_CORES  # 2048 samples per core
NBLK = NL // 128  # 64 location blocks
MM_N = 512  # fp32 moving-operand limit
BW = 0.5
INV_BW2 = 1.0 / (BW * BW)  # 4.0
C2 = 2.0 * INV_BW2  # 8.0
PEN = 500.0
FOLD = 2.0 * PEN
N_CHUNKS = 4  # all-reduce chunks overlapped with compute

_STATE = {}


def build_nc():
    import concourse.bacc as bacc
    import concourse.mybir as mybir
    import concourse.tile as tile

    f32 = mybir.dt.float32
    AX = mybir.AxisListType
    AF = mybir.ActivationFunctionType
    AL = mybir.AluOpType

    nc = bacc.Bacc(None, target_bir_lowering=False, num_devices=N_CORES)

    bf16 = mybir.dt.bfloat16
    s_t = nc.declare_dram_parameter("samples_t", [2, NS_SH], f32, isOutput=False)
    l_s = nc.declare_dram_parameter("loc_split", [6, NL], bf16, isOutput=False)
    l_n = nc.declare_dram_parameter("locations_n", [128, 2 * NBLK], f32, isOutput=False)
    out_d = nc.declare_dram_parameter("out", [128, NBLK], f32, isOutput=True)

    with tile.TileContext(nc) as tc:
        with tc.tile_pool(name="const", bufs=1) as cpool, \
             tc.tile_pool(name="dram", bufs=1, space="DRAM") as dpool, \
             tc.tile_pool(name="escr", bufs=2) as epool, \
             tc.tile_pool(name="ps", bufs=2, space="PSUM") as ppool:

            bf = bf16
            # stationary rows: [1 x6; lxh; lyh; lxh; lyh; lxl; lyl]
            Lb = cpool.tile([12, NL], bf)
            LL = cpool.tile([128, 2 * NBLK], f32)  # [lx | ly] natural
            S2 = cpool.tile([2, NS_SH], f32)  # [sx; sy]
            # moving rows: [penx;peny; thx;thy; tlx;tly; xh;yh; xl;yl; xh;yh]
            Rb = cpool.tile([12, NS_SH], bf)
            R8 = cpool.tile([2, NS_SH], f32)  # 8*S2 (base partition 0)
            hi2 = cpool.tile([2, NS_SH], bf)
            lo2 = cpool.tile([2, NS_SH], bf)
            tf2 = cpool.tile([2, NS_SH], f32)
            th2b = cpool.tile([2, NS_SH], bf)
            tl2b = cpool.tile([2, NS_SH], bf)
            pen2b = cpool.tile([2, NS_SH], bf)
            mt = cpool.tile([1, 2], f32)  # (mx, my) at partition 0
            B = cpool.tile([128, NBLK], f32)  # ACT bias
            acc = cpool.tile([128, NBLK], f32)  # partial kernel sums
            m2 = cpool.tile([2, 1], f32)  # (mx, my) bbox bounds
            sq = cpool.tile([2, NS_SH], f32)
            A2 = cpool.tile([2, NS_SH], f32)
            U = cpool.tile([2, NS_SH], f32)
            rm = cpool.tile([128, 2], f32)
            t1 = cpool.tile([128, NBLK], f32)
            t2 = cpool.tile([128, NBLK], f32)
            G = cpool.tile([128, NBLK], f32)
            Gs = cpool.tile([128, 1], f32)
            tot = cpool.tile([1, 1], f32)
            rtot = cpool.tile([1, 1], f32)
            rb = cpool.tile([128, 1], f32)
            ones1 = cpool.tile([1, 128], f32)

            # uneven chunks: small final chunk minimizes the exposed tail
            BNDS = [0, 20, 40, 56, 64]
            partials = [
                dpool.tile([128, BNDS[g + 1] - BNDS[g]], f32, name=f"partial{g}")
                for g in range(N_CHUNKS)
            ]
            allsums = [
                dpool.tile(
                    [128, BNDS[g + 1] - BNDS[g]],
                    f32,
                    addr_space="Shared",
                    name=f"allsum{g}",
                )
                for g in range(N_CHUNKS)
            ]

            # ---- input loads (all contiguous) ----
            nc.gpsimd.memset(Lb[0:6, :], 1.0)
            nc.sync.dma_start(out=Lb[6:12, :], in_=l_s[:, :])
            nc.sync.dma_start(out=LL[:, :], in_=l_n[:, :])
            nc.sync.dma_start(out=S2[:, :], in_=s_t[:, :])

            lx = LL[:, 0:NBLK]
            ly = LL[:, NBLK : 2 * NBLK]

            # ---- location-side prep: bias and bbox bounds ----
            nc.vector.tensor_tensor(t1[:], lx, lx, AL.mult)
            nc.vector.tensor_tensor(t2[:], ly, ly, AL.mult)
            nc.vector.tensor_tensor(t1[:], t1[:], t2[:], AL.add)
            nc.vector.tensor_scalar(B[:], t1[:], -INV_BW2, None, AL.mult)

            nc.vector.tensor_reduce(
                rm[:, 0:1], lx, axis=AX.X, op=AL.max, apply_absolute_value=True
            )
            nc.vector.tensor_reduce(
                rm[:, 1:2], ly, axis=AX.X, op=AL.max, apply_absolute_value=True
            )
            nc.gpsimd.tensor_reduce(mt[:, :], rm[:, :], axis=AX.C, op=AL.max)
            # scatter (mx, my) to partitions 0 and 1 (DMA has no base restriction)
            nc.sync.dma_start(out=m2[0:1, :], in_=mt[:, 0:1])
            nc.sync.dma_start(out=m2[1:2, :], in_=mt[:, 1:2])

            # ---- sample-side prep (all compute at base partition 0) ----
            # hi/lo bf16 split of 8*s so the matmul can run in bf16 while
            # keeping ~f32 accuracy (hi*hi, hi*lo, lo*hi products, f32 PSUM).
            # Per-coordinate t and pen rows pair with ones-rows in the
            # stationary, so no cross-partition folds are needed.
            nc.vector.tensor_scalar(R8[:], S2[:], C2, None, AL.mult)
            nc.vector.tensor_copy(hi2[:], R8[:])
            nc.vector.tensor_tensor(lo2[:], R8[:], hi2[:], AL.subtract)
            nc.vector.tensor_tensor(sq[:], S2[:], S2[:], AL.mult)
            nc.scalar.activation(A2[:], S2[:], AF.Abs)
            # t = -4*s^2 split into th+tl (bf16 pair per coordinate)
            nc.vector.tensor_scalar(tf2[:], sq[:], -INV_BW2, None, AL.mult)
            nc.vector.tensor_copy(th2b[:], tf2[:])
            nc.vector.tensor_tensor(tl2b[:], tf2[:], th2b[:], AL.subtract)
            # pen = 500*(|s| < m) - 500 per coordinate (exact bf16 values)
            nc.vector.tensor_scalar(U[:], A2[:], m2[:, 0:1], None, AL.is_lt)
            nc.vector.tensor_scalar(pen2b[:], U[:], PEN, -PEN, AL.mult, AL.add)
            # assemble moving operand (DMA may write any base partition)
            nc.sync.dma_start(out=Rb[0:2, :], in_=pen2b[:])
            nc.sync.dma_start(out=Rb[2:4, :], in_=th2b[:])
            nc.sync.dma_start(out=Rb[4:6, :], in_=tl2b[:])
            nc.sync.dma_start(out=Rb[6:8, :], in_=hi2[:])
            nc.sync.dma_start(out=Rb[8:10, :], in_=lo2[:])
            nc.sync.dma_start(out=Rb[10:12, :], in_=hi2[:])

            # ---- main loop: 64 location blocks, chunked all-reduce overlap ----
            for b in range(NBLK):
                ps = ppool.tile([128, NS_SH], f32, tag="ps")
                for n in range(NS_SH // MM_N):
                    nc.tensor.matmul(
                        ps[:, n * MM_N : (n + 1) * MM_N],
                        lhsT=Lb[:, b * 128 : (b + 1) * 128],
                        rhs=Rb[:, n * MM_N : (n + 1) * MM_N],
                        start=True,
                        stop=True,
                    )
                es = epool.tile([128, NS_SH], f32, tag="es")
                nc.scalar.activation(
                    es[:],
                    ps[:],
                    AF.Exp,
                    bias=B[:, b : b + 1],
                    scale=1.0,
                    accum_out=acc[:, b : b + 1],
                )
                if b + 1 in BNDS:
                    g = BNDS.index(b + 1) - 1
                    lo, hi = BNDS[g], BNDS[g + 1]
                    nc.sync.dma_start(
                        out=partials[g][:, :], in_=acc[:, lo:hi]
                    )
                    nc.gpsimd.collective_compute(
                        "AllReduce",
                        AL.add,
                        replica_groups=[list(range(N_CORES))],
                        ins=[partials[g][:, :]],
                        outs=[allsums[g][:, :]],
                    )

            # ---- normalize on-device ----
            for g in range(N_CHUNKS):
                nc.sync.dma_start(
                    out=G[:, BNDS[g] : BNDS[g + 1]], in_=allsums[g][:, :]
                )
            nc.vector.tensor_reduce(Gs[:], G[:], axis=AX.X, op=AL.add)
            nc.gpsimd.tensor_reduce(tot[:], Gs[:], axis=AX.C, op=AL.add)
            nc.vector.reciprocal(rtot[:], tot[:])
            # broadcast 1/norm to all 128 partitions via PE (ones is LT row 2)
            psb = ppool.tile([128, 1], f32, tag="ps")
            nc.gpsimd.memset(ones1[:], 1.0)
            nc.tensor.matmul(
                psb[:], lhsT=ones1[:], rhs=rtot[:], start=True, stop=True
            )
            nc.scalar.copy(rb[:], psb[:])
            nc.vector.tensor_scalar(G[:], G[:], rb[:], None, AL.mult)
            nc.sync.dma_start(out=out_d[:, :], in_=G[:])

    nc.compile()  # Bacc register allocation / DCE — required before walrus
    return nc


def _loc_layouts(locations):
    from ml_dtypes import bfloat16

    # block-permuted transpose: column b*128+p holds location j = p*64+b
    lt = np.ascontiguousarray(
        locations.T.reshape(2, 128, NBLK).transpose(0, 2, 1).reshape(2, NL)
    )
    # hi/lo bf16 split (lossless re-encoding of the f32 coords; rows are
    # [lxh, lyh, lxh, lyh, lxl, lyl] matching the K=9 stationary layout)
    lth = lt.astype(bfloat16)
    ltl = (lt - lth.astype(np.float32)).astype(bfloat16)
    ls = np.ascontiguousarray(np.concatenate([lth, lth, ltl], axis=0))
    # locations_n: [128, 128], cols 0..63 = lx, 64..127 = ly, row p / col b = j=p*64+b
    ln3 = locations.reshape(128, NBLK, 2)
    ln = np.ascontiguousarray(
        np.concatenate([ln3[:, :, 0], ln3[:, :, 1]], axis=1)
    )
    return ls, ln


def make_in_maps(samples, locations):
    ls, ln = _loc_layouts(locations)
    in_maps = []
    for c in range(N_CORES):
        shard = samples[c * NS_SH : (c + 1) * NS_SH]
        in_maps.append(
            {
                "samples_t": np.ascontiguousarray(shard.T),
                "loc_split": ls,
                "locations_n": ln,
            }
        )
    return in_maps


def kernel(samples, locations):
    samples = np.ascontiguousarray(np.asarray(samples, dtype=np.float32))
    locations = np.ascontiguousarray(np.asarray(locations, dtype=np.float32))
    assert samples.shape == (NS, 2) and locations.shape == (NL, 2)

    from concourse.bass_utils import run_bass_kernel_spmd

    if "nc" not in _STATE:
        _STATE["nc"] = build_nc()
    nc = _STATE["nc"]

    in_maps = make_in_maps(samples, locations)
    res = run_bass_kernel_spmd(
        nc,
        in_maps,
        list(range(N_CORES)),
        trace=bool(_STATE.get("trace", False)),
    )
    _STATE["exec_time_ns"] = res.exec_time_ns
    _STATE["profile_json"] = res.profile_json
    return np.asarray(res.results[0]["out"], dtype=np.float32).reshape(NL)



# revision 15
# speedup vs baseline: 1.9139x; 1.9139x over previous
"""Gaussian KDE (bandwidth=0.5) on 8 TRN2 NeuronCores — grid-factorized.

out[j] = sum_i mask_i * exp(-|s_i - l_j|^2 / bw^2), normalized to sum 1.

Algorithm (exact Gaussian-lattice factorization, NOT an approximation knob):
  exp(-|s-l|^2/(2v)) with v = bw^2/2 = 0.125 factorizes over a uniform grid
  g_u = h*c_u (c_u = u-63.5, h = 2M/119, M = per-axis abs-max of locations):

      sum_u exp(-(s-g_u)^2/(2h^2)) * exp(-(g_u-l)^2/(2v'))
        = C * exp(-(s-l)^2/(2(v'+h^2)))        [Gaussian o Gaussian, exact]
  with v' = v - h^2.  The lattice-sum constant C is independent of s up to
  a Poisson ripple exp(-2 pi^2) ~ 5e-9, and cancels in the normalization.

  So per core (samples sharded 8-way, locations sharded 8-way):
    Wx[i,u] = exp(-(sx_i-g_u)^2/(2h^2))   (x-window), same Wy     [2048 x 128]
    Ht[v,u] = sum_i Wy[i,v]*Wx[i,u]       (PE, partial over sample shard)
    P[j,u]  = exp(-a'(gx_u-lx_j)^2), Q[j,v] = exp(-a'(gy_v-ly_j)^2),
              a' = 1/(2 v')               (location shard, 1024 locs)
    T2[v,u] = sum_j Q[j,v]*P[j,u]         (PE, partial over location shard)
    ONE AllReduce of [Ht | T2]  (128x256 f32)
    R[j,u]  = sum_v Qt[v,j]*Ht[v,u]       (PE)
    out[j]  = sum_u P[j,u]*R[j,u],  norm = sum_{v,u} Ht*T2  (= sum_j out_j)
    out /= norm  (on device)

  Samples outside the location bbox (strict |s| < M per axis, torch mask
  semantics) are pushed +1000 before binning -> their window underflows to 0.

Engine plan: ScalarE runs ONLY Exp (no act-table switches); DVE+GpSimd build
the quadratic exp arguments with tensor_scalar/scalar_tensor_tensor; PE does
the three contractions in bf16 (operands are exps in [0,1]; rel err ~1e-3).
"""

import sys

sys.path.insert(0, "/opt/trn_rl_repo")

import numpy as np

N_CORES = 8
NS = 16384
NL = 8192
NS_SH = NS // N_CORES  # 2048 samples per core
NL_SH = NL // N_CORES  # 1024 locations per core
G = 128  # grid nodes per axis
NSB = NS_SH // 128  # 16 sample blocks
NLB = NL_SH // 128  # 8 location blocks
GDEN = 119.0  # grid half-width = M * 127/119ish margin (4h pad for windows)
V = 0.125  # bw^2 / 2

_STATE = {}


def build_nc():
    import concourse.bacc as bacc
    import concourse.mybir as mybir
    import concourse.tile as tile
    from concourse import bass_isa

    f32 = mybir.dt.float32
    bf16 = mybir.dt.bfloat16
    AX = mybir.AxisListType
    AF = mybir.ActivationFunctionType
    AL = mybir.AluOpType
    RO = bass_isa.ReduceOp

    nc = bacc.Bacc(None, target_bir_lowering=False, num_devices=N_CORES)

    s_cols = nc.declare_dram_parameter("s_cols", [128, 2 * NSB], f32, isOutput=False)
    l_xc = nc.declare_dram_parameter("l_xcols", [128, NLB], f32, isOutput=False)
    l_yc = nc.declare_dram_parameter("l_ycols", [128, NLB], f32, isOutput=False)
    l_yr = nc.declare_dram_parameter("l_yrow", [1, NL_SH], f32, isOutput=False)
    l_all = nc.declare_dram_parameter("l_all", [128, 128], f32, isOutput=False)
    iot_d = nc.declare_dram_parameter("iota_cb", [128, 2 * G], f32, isOutput=False)
    col_d = nc.declare_dram_parameter("colc", [128, 1], f32, isOutput=False)
    out_d = nc.declare_dram_parameter("out", [128, NLB], f32, isOutput=True)

    with tile.TileContext(nc) as tc:
        with tc.tile_pool(name="const", bufs=1) as cpool, \
             tc.tile_pool(name="dram", bufs=1, space="DRAM") as dpool, \
             tc.tile_pool(name="wa", bufs=3) as wapool, \
             tc.tile_pool(name="wexp", bufs=4) as wepool, \
             tc.tile_pool(name="ps", bufs=1, space="PSUM") as ppool:

            SC = cpool.tile([128, 2 * NSB], f32)  # sample cols [sx | sy]
            LXC = cpool.tile([128, NLB], f32)
            LYC = cpool.tile([128, NLB], f32)
            LYR = cpool.tile([1, NL_SH], f32)
            LA = cpool.tile([128, 128], f32)
            IOT = cpool.tile([128, 2 * G], f32)  # c_u both halves
            COLC = cpool.tile([128, 1], f32)  # c_p per partition

            rm = cpool.tile([128, 2], f32)
            Mb = cpool.tile([128, 2], f32)
            h = cpool.tile([128, 2], f32)
            rh = cpool.tile([128, 2], f32)
            hsq = cpool.tile([128, 2], f32)
            vp = cpool.tile([128, 2], f32)
            rvp = cpool.tile([128, 2], f32)
            na = cpool.tile([128, 2], f32)  # -a' per axis
            gqc = cpool.tile([128, 1], f32)  # gy_v = h_y * c_v

            nSC = cpool.tile([128, 2 * NSB], f32)
            U4 = cpool.tile([128, 4 * NSB], f32)
            Ux = cpool.tile([128, NSB], f32)
            Uy = cpool.tile([128, NSB], f32)
            msk = cpool.tile([128, NSB], f32)
            pm = cpool.tile([128, NSB], f32)
            spx = cpool.tile([128, NSB], f32)
            spy = cpool.tile([128, NSB], f32)
            zx = cpool.tile([128, NSB], f32)
            zy = cpool.tile([128, NSB], f32)

            GP = cpool.tile([128, 2 * G], f32)  # [gx_u | gy_u]
            LYB = cpool.tile([128, NL_SH], f32)
            QD = cpool.tile([128, NL_SH], f32)
            QS = cpool.tile([128, NL_SH], f32)
            Qt = cpool.tile([128, NL_SH], f32)
            PQE = [cpool.tile([128, 2 * G], f32, name=f"pqe{q}") for q in range(NLB)]

            CCS = cpool.tile([128, 2 * G], f32)
            HTg = cpool.tile([128, 2 * G], f32)
            ACC = cpool.tile([128, NLB], f32)
            scr = cpool.tile([128, G], f32)
            scr2 = cpool.tile([128, G], f32)
            ns_ = cpool.tile([128, 1], f32)
            ntb = cpool.tile([128, 1], f32)
            rtot = cpool.tile([128, 1], f32)
            OUT = cpool.tile([128, NLB], f32)

            cc_in = dpool.tile([128, 2 * G], f32, name="cc_in")
            cc_out = dpool.tile([128, 2 * G], f32, addr_space="Shared", name="cc_out")

            Ht_ps = ppool.tile([128, G], f32, tag="ht")
            T2_ps = ppool.tile([128, G], f32, tag="t2")
            R_ps = ppool.tile([128, NL_SH], f32, tag="r")

            # ---- input loads ----
            nc.sync.dma_start(out=SC[:, :], in_=s_cols[:, :])
            nc.sync.dma_start(out=LXC[:, :], in_=l_xc[:, :])
            nc.sync.dma_start(out=LYC[:, :], in_=l_yc[:, :])
            nc.sync.dma_start(out=LYR[:, :], in_=l_yr[:, :])
            nc.sync.dma_start(out=LA[:, :], in_=l_all[:, :])
            nc.sync.dma_start(out=IOT[:, :], in_=iot_d[:, :])
            nc.sync.dma_start(out=COLC[:, :], in_=col_d[:, :])

            # ---- bbox bounds M (global over all 8192 locations) ----
            nc.vector.tensor_reduce(
                rm[:, 0:1], LA[:, 0:64], axis=AX.X, op=AL.max,
                apply_absolute_value=True,
            )
            nc.vector.tensor_reduce(
                rm[:, 1:2], LA[:, 64:128], axis=AX.X, op=AL.max,
                apply_absolute_value=True,
            )
            nc.gpsimd.partition_all_reduce(Mb[:, :], rm[:, :], 128, RO.max)

            # ---- runtime scalars (all [128,2] broadcast, x col 0 / y col 1) ----
            nc.vector.tensor_scalar(h[:], Mb[:], 2.0 / GDEN, None, AL.mult)
            nc.vector.reciprocal(rh[:], h[:])
            nc.vector.tensor_tensor(hsq[:], h[:], h[:], AL.mult)
            nc.vector.tensor_scalar(vp[:], hsq[:], -1.0, V, AL.mult, AL.add)
            nc.vector.reciprocal(rvp[:], vp[:])
            nc.vector.tensor_scalar(na[:], rvp[:], -0.5, None, AL.mult)
            nc.vector.tensor_scalar(gqc[:], COLC[:], h[:, 1:2], None, AL.mult)

            # ---- sample prep: mask + z = s/h  ([128, NSB] col k = block) ----
            nc.vector.tensor_scalar(nSC[:], SC[:], -1.0, None, AL.mult)
            nc.vector.tensor_scalar(
                U4[:, 0:NSB], SC[:, 0:NSB], Mb[:, 0:1], None, AL.is_lt
            )
            nc.vector.tensor_scalar(
                U4[:, NSB : 2 * NSB], nSC[:, 0:NSB], Mb[:, 0:1], None, AL.is_lt
            )
            nc.vector.tensor_scalar(
                U4[:, 2 * NSB : 3 * NSB], SC[:, NSB : 2 * NSB], Mb[:, 1:2], None,
                AL.is_lt,
            )
            nc.vector.tensor_scalar(
                U4[:, 3 * NSB : 4 * NSB], nSC[:, NSB : 2 * NSB], Mb[:, 1:2], None,
                AL.is_lt,
            )
            nc.vector.tensor_tensor(
                Ux[:], U4[:, 0:NSB], U4[:, NSB : 2 * NSB], AL.mult
            )
            nc.vector.tensor_tensor(
                Uy[:], U4[:, 2 * NSB : 3 * NSB], U4[:, 3 * NSB : 4 * NSB], AL.mult
            )
            nc.vector.tensor_tensor(msk[:], Ux[:], Uy[:], AL.mult)
            nc.vector.tensor_scalar(pm[:], msk[:], -1000.0, 1000.0, AL.mult, AL.add)
            nc.vector.tensor_tensor(spx[:], SC[:, 0:NSB], pm[:], AL.add)
            nc.vector.tensor_tensor(spy[:], SC[:, NSB : 2 * NSB], pm[:], AL.add)
            nc.vector.tensor_scalar(zx[:], spx[:], rh[:, 0:1], None, AL.mult)
            nc.vector.tensor_scalar(zy[:], spy[:], rh[:, 1:2], None, AL.mult)

            # ---- eval grid GP = h*c (unscaled coords) ----
            nc.vector.tensor_scalar(
                GP[:, 0:G], IOT[:, 0:G], h[:, 0:1], None, AL.mult
            )
            nc.vector.tensor_scalar(
                GP[:, G : 2 * G], IOT[:, G : 2 * G], h[:, 1:2], None, AL.mult
            )

            # ---- Qt[v, j] = exp(-a'_y (gy_v - ly_j)^2)  [128, 1024] ----
            nc.gpsimd.partition_broadcast(LYB[:, :], LYR[0:1, :], 128)
            nc.vector.tensor_scalar(QD[:], LYB[:], gqc[:, 0:1], None, AL.subtract)
            nc.vector.scalar_tensor_tensor(
                QS[:], QD[:], na[:, 1:2], QD[:], AL.mult, AL.mult
            )
            nc.scalar.activation(Qt[:], QS[:], AF.Exp)

            # ---- binning: W[i, u|v] windows, Ht += Wy^T Wx  (PE bf16) ----
            for k in range(NSB):
                eng = nc.vector
                D = wapool.tile([128, 2 * G], f32, tag="wd")
                SQ = wapool.tile([128, 2 * G], f32, tag="wsq")
                eng.tensor_scalar(
                    D[:, 0:G], IOT[:, 0:G], zx[:, k : k + 1], None,
                    AL.subtract,
                )
                eng.tensor_scalar(
                    D[:, G : 2 * G], IOT[:, G : 2 * G], zy[:, k : k + 1], None,
                    AL.subtract,
                )
                eng.scalar_tensor_tensor(SQ[:], D[:], -0.5, D[:], AL.mult, AL.mult)
                W = wepool.tile([128, 2 * G], f32, tag="we")
                nc.scalar.activation(W[:], SQ[:], AF.Exp)
                nc.tensor.matmul(
                    Ht_ps[:, :],
                    lhsT=W[:, G : 2 * G],
                    rhs=W[:, 0:G],
                    start=(k == 0),
                    stop=(k == NSB - 1),
                )

            # ---- P/Q eval tiles + T2 += Q^T P  (location shard) ----
            for q in range(NLB):
                eng = nc.vector
                D = wapool.tile([128, 2 * G], f32, tag="wd")
                SQ = wapool.tile([128, 2 * G], f32, tag="wsq")
                eng.tensor_scalar(
                    D[:, 0:G], GP[:, 0:G], LXC[:, q : q + 1], None, AL.subtract
                )
                eng.tensor_scalar(
                    D[:, G : 2 * G], GP[:, G : 2 * G], LYC[:, q : q + 1], None,
                    AL.subtract,
                )
                eng.scalar_tensor_tensor(
                    SQ[:, 0:G], D[:, 0:G], na[:, 0:1], D[:, 0:G], AL.mult, AL.mult
                )
                eng.scalar_tensor_tensor(
                    SQ[:, G : 2 * G], D[:, G : 2 * G], na[:, 1:2], D[:, G : 2 * G],
                    AL.mult, AL.mult,
                )
                nc.scalar.activation(PQE[q][:], SQ[:], AF.Exp)
                nc.tensor.matmul(
                    T2_ps[:, :],
                    lhsT=PQE[q][:, G : 2 * G],
                    rhs=PQE[q][:, 0:G],
                    start=(q == 0),
                    stop=(q == NLB - 1),
                )

            # ---- pack + single AllReduce of [Ht | T2] ----
            nc.vector.tensor_copy(CCS[:, 0:G], Ht_ps[:, :])
            nc.scalar.copy(CCS[:, G : 2 * G], T2_ps[:, :])
            nc.sync.dma_start(out=cc_in[:, :], in_=CCS[:, :])
            nc.gpsimd.collective_compute(
                "AllReduce",
                AL.add,
                replica_groups=[list(range(N_CORES))],
                ins=[cc_in[:, :]],
                outs=[cc_out[:, :]],
            )
            nc.sync.dma_start(out=HTg[:, :], in_=cc_out[:, :])

            # ---- R[j,u] = sum_v Qt[v,j] * Ht[v,u]; out_j = sum_u P*R ----
            for q in range(NLB):
                nc.tensor.matmul(
                    R_ps[:, q * G : (q + 1) * G],
                    lhsT=Qt[:, q * G : (q + 1) * G],
                    rhs=HTg[:, 0:G],
                    start=True,
                    stop=True,
                )
                nc.vector.scalar_tensor_tensor(
                    scr[:, :],
                    PQE[q][:, 0:G],
                    1.0,
                    R_ps[:, q * G : (q + 1) * G],
                    AL.mult,
                    AL.mult,
                    accum_out=ACC[:, q : q + 1],
                )

            # ---- norm = sum_{v,u} Ht*T2 ; out /= norm ----
            nc.vector.scalar_tensor_tensor(
                scr2[:, :],
                HTg[:, 0:G],
                1.0,
                HTg[:, G : 2 * G],
                AL.mult,
                AL.mult,
                accum_out=ns_[:, :],
            )
            nc.gpsimd.partition_all_reduce(ntb[:, :], ns_[:, :], 128, RO.add)
            nc.vector.reciprocal(rtot[:], ntb[:])
            nc.vector.tensor_scalar(OUT[:], ACC[:], rtot[:, 0:1], None, AL.mult)
            nc.sync.dma_start(out=out_d[:, :], in_=OUT[:])

    nc.compile()
    return nc


def make_in_maps(samples, locations):
    sx = samples[:, 0].reshape(N_CORES, NSB, 128)
    sy = samples[:, 1].reshape(N_CORES, NSB, 128)
    lx = locations[:, 0].reshape(N_CORES, NLB, 128)
    ly = locations[:, 1].reshape(N_CORES, NLB, 128)
    la = np.concatenate(
        [
            np.ascontiguousarray(locations[:, 0].reshape(64, 128).T),
            np.ascontiguousarray(locations[:, 1].reshape(64, 128).T),
        ],
        axis=1,
    ).astype(np.float32)
    c = (np.arange(G, dtype=np.float32) - 63.5)
    iota_cb = np.tile(np.concatenate([c, c])[None, :], (128, 1))
    colc = (np.arange(128, dtype=np.float32) - 63.5)[:, None]
    in_maps = []
    for cid in range(N_CORES):
        s_cols = np.concatenate(
            [sx[cid].T, sy[cid].T], axis=1
        )  # [128, 32]: col k = block k
        l_xcols = lx[cid].T  # [128, 8] col q part m = shard[q*128+m]
        l_ycols = ly[cid].T
        l_yrow = ly[cid].reshape(1, NL_SH)
        in_maps.append(
            {
                "s_cols": np.ascontiguousarray(s_cols, dtype=np.float32),
                "l_xcols": np.ascontiguousarray(l_xcols, dtype=np.float32),
                "l_ycols": np.ascontiguousarray(l_ycols, dtype=np.float32),
                "l_yrow": np.ascontiguousarray(l_yrow, dtype=np.float32),
                "l_all": np.ascontiguousarray(la, dtype=np.float32),
                "iota_cb": np.ascontiguousarray(iota_cb, dtype=np.float32),
                "colc": np.ascontiguousarray(colc, dtype=np.float32),
            }
        )
    return in_maps


def kernel(samples, locations):
    samples = np.ascontiguousarray(np.asarray(samples, dtype=np.float32))
    locations = np.ascontiguousarray(np.asarray(locations, dtype=np.float32))
    assert samples.shape == (NS, 2) and locations.shape == (NL, 2)

    from concourse.bass_utils import run_bass_kernel_spmd

    if "nc" not in _STATE:
        _STATE["nc"] = build_nc()
    nc = _STATE["nc"]

    in_maps = make_in_maps(samples, locations)
    res = run_bass_kernel_spmd(
        nc,
        in_maps,
        list(range(N_CORES)),
        trace=bool(_STATE.get("trace", False)),
    )
    _STATE["exec_time_ns"] = res.exec_time_ns
    _STATE["profile_json"] = res.profile_json
    outs = [
        np.asarray(res.results[c]["out"], dtype=np.float32).T.reshape(NL_SH)
        for c in range(N_CORES)
    ]
    return np.concatenate(outs)
